# revision 33
# baseline (speedup 1.0000x reference)
"""Segment-mean kernel for nn_AttentionedSumLayer (Trainium2, 8 NeuronCores).

The reference's score chain is dead code (exp scores are overwritten with
ones), so the computation reduces to a segment mean over token rows:
    out[n, :] = mean(data[i, :] for i with tokens_to_node_map[i] == n)
with out[n] = 0 for empty nodes.  data is (1M, 256) f32, 100k nodes.

Strategy (memory-bound; ~1.1 GB must stream from HBM):
  * Host: sort tokens by node id; nodes grouped into 782 blocks of 128.
    Groups are dealt to the 8 cores balanced by tile count so every core
    compiles to the identical static schedule (true SPMD).
  * Host arranges each core's token rows into one contiguous [128, T*256]
    stream: per node-group a [128, S_j*256] chunk (partition p holds S_j
    consecutive token rows), so each group loads with a single ~1.4 MB DMA.
  * Device: per group, S_j matmuls accumulate one-hot(node)ᵀ @ data_tile
    into a [128 nodes, 256] PSUM tile.  The one-hot is built on the fly by
    comparing an iota row against the per-token relative node id (padding
    rows get -1 and vanish).  PSUM is scaled by 1/count on the ACT engine
    and streamed out.
"""

import math
import os

import numpy as np

NUM_NODES = 100000
N_CORES = 8
P = 128
F = 256

# module-level knobs (test.py pokes these; harness uses defaults)
# MODE: 'f32' exact fp32 matmuls; 'bf16' bf16 data (relmax ~2e-3, halves DMA);
#       'split' bf16 hi+lo streams (relmax ~4e-6, same DMA as f32, faster PE)
TRACE = os.environ.get("BASS_PROBLEM_TRACE", "") == "1"
MODE = os.environ.get("BASS_PROBLEM_MODE", "split")
LAST_RESULTS = None  # BassKernelResults of the last run (for test.py)


# ---------------------------------------------------------------------------
# workaround: this walrus build rejects instructions carrying more than one
# sem wait ("Too many sync wait commands", CoreV*GenImpl setupSyncWait).
# After Tile scheduling, hoist excess waits onto same-engine NoOps inserted
# immediately before the over-limit instruction (waits only delay, so moving
# them earlier on the same engine is sound).
_MAX_WAITS = 1


def _split_waits(nc):
    import concourse.mybir as mybir

    uid = 0
    for f in nc.m.functions:
        for bb in f.blocks:
            out = []
            for inst in bb.instructions:
                si = inst.sync_info
                if si is not None and len(si.on_wait) > _MAX_WAITS:
                    waits = list(si.on_wait)
                    extra, keep = waits[:-_MAX_WAITS], waits[-_MAX_WAITS:]
                    for i in range(0, len(extra), _MAX_WAITS):
                        nop = mybir.InstNoOp(
                            name=f"wsplit-{uid}", engine=inst.engine
                        )
                        uid += 1
                        nop.sync_info = mybir.SyncInfo(
                            on_wait=extra[i : i + _MAX_WAITS], on_update=[]
                        )
                        out.append(nop)
                    si.on_wait = keep
                out.append(inst)
            bb.instructions = out


# ---------------------------------------------------------------------------
def _enable_profiling():
    """Best-effort: register the axon NTFF profile hook shim so trace=True
    works (antenv.axon_hooks is absent in this image) and stub the fish
    artifact upload.  Returns True when profiling is available."""
    try:
        import sys, types

        from trn_agent_boot.trn_boot import _ntff_profile_via_ctypes
        from concourse import bass_utils

        if "antenv.axon_hooks" not in sys.modules:
            hook = _ntff_profile_via_ctypes("/opt/axon/libaxon_pjrt.so")
            if hook is None:
                return False
            mod = types.ModuleType("antenv.axon_hooks")
            mod.get_axon_ntff_profile_hook = lambda: hook
            sys.modules["antenv.axon_hooks"] = mod
        bass_utils.upload_artifacts = lambda tmpdir: f"local://{tmpdir}"
        return True
    except Exception:
        return False


# ---------------------------------------------------------------------------
def _preprocess(data, tokens_map):
    """Sort/arrange full inputs into per-core SPMD-uniform streams."""
    m = np.asarray(tokens_map).astype(np.int64).ravel()
    data = np.ascontiguousarray(np.asarray(data, dtype=np.float32))
    n_tok = m.shape[0]

    counts = np.bincount(m, minlength=NUM_NODES)
    inv = np.zeros(NUM_NODES, np.float32)
    nz = counts > 0
    inv[nz] = 1.0 / counts[nz]

    order = np.argsort(m, kind="stable")
    sorted_nodes = m[order]

    n_groups = math.ceil(NUM_NODES / P)  # 782
    grp_bounds = np.searchsorted(sorted_nodes, np.arange(n_groups + 1) * P)
    grp_tok = np.diff(grp_bounds)
    tiles_g = np.maximum(1, -(-grp_tok // P))  # ceil, min 1

    # deal groups to cores: sort by tile count desc, position j takes the
    # next 8; every core's position-j group is padded to the max of that
    # block so all cores share one static schedule.
    sort_idx = np.argsort(-tiles_g, kind="stable")
    n_pos = math.ceil(n_groups / N_CORES)  # 98
    S = np.zeros(n_pos, np.int64)
    assign = np.full((N_CORES, n_pos), -1, np.int64)
    for j in range(n_pos):
        blk = sort_idx[N_CORES * j : N_CORES * (j + 1)]
        S[j] = tiles_g[blk[0]]
        for c, g in enumerate(blk):
            assign[c, j] = g
    T_core = int(S.sum())

    if MODE in ("bf16", "split"):
        import ml_dtypes

        bf16 = ml_dtypes.bfloat16

    in_maps = []
    for c in range(N_CORES):
        if MODE == "f32":
            streams = {"data": np.zeros((P, T_core * F), np.float32)}
        elif MODE == "bf16":
            streams = {"data": np.zeros((P, T_core * F), bf16)}
        else:  # split
            streams = {
                "data": np.zeros((P, T_core * F), bf16),
                "data_lo": np.zeros((P, T_core * F), bf16),
            }
        rel_dt = np.float32 if MODE == "f32" else bf16
        rel = np.full((P, T_core), -1.0, rel_dt)
        invm = np.zeros((P, n_pos), np.float32)
        t0 = 0
        for j in range(n_pos):
            Sj = int(S[j])
            g = int(assign[c, j])
            if g >= 0:
                toks = order[grp_bounds[g] : grp_bounds[g + 1]]
                n = len(toks)
                L = P * Sj
                blk = np.zeros((L, F), np.float32)
                blk[:n] = data[toks]
                blk2 = blk.reshape(P, Sj * F)
                if MODE == "f32":
                    streams["data"][:, t0 * F : (t0 + Sj) * F] = blk2
                elif MODE == "bf16":
                    streams["data"][:, t0 * F : (t0 + Sj) * F] = blk2.astype(bf16)
                else:
                    hi = blk2.astype(bf16)
                    streams["data"][:, t0 * F : (t0 + Sj) * F] = hi
                    streams["data_lo"][:, t0 * F : (t0 + Sj) * F] = (
                        blk2 - hi.astype(np.float32)
                    ).astype(bf16)
                relblk = np.full(L, -1.0, rel_dt)
                relblk[:n] = (m[toks] - P * g).astype(rel_dt)
                rel[:, t0 : t0 + Sj] = relblk.reshape(P, Sj)
                base = P * g
                nvalid = min(P, NUM_NODES - base)
                invm[:nvalid, j] = inv[base : base + nvalid]
            t0 += Sj
        streams["rel"] = rel
        streams["invc"] = invm
        in_maps.append(streams)

    meta = {"S": S, "assign": assign, "n_pos": n_pos, "T_core": T_core}
    return in_maps, meta


# ---------------------------------------------------------------------------
def _build_kernel(S, n_pos, T_core):
    import concourse.bass as bass
    import concourse.mybir as mybir
    from concourse.tile import TileContext

    f32 = mybir.dt.float32
    mm_dt = f32 if MODE == "f32" else mybir.dt.bfloat16

    nc = bass.Bass()
    data_d = nc.dram_tensor("data", (P, T_core * F), mm_dt, kind="ExternalInput")
    lo_d = None
    if MODE == "split":
        lo_d = nc.dram_tensor(
            "data_lo", (P, T_core * F), mm_dt, kind="ExternalInput"
        )
    rel_d = nc.dram_tensor("rel", (P, T_core), mm_dt, kind="ExternalInput")
    inv_d = nc.dram_tensor("invc", (P, n_pos), f32, kind="ExternalInput")
    out_dt = f32 if MODE == "f32" else mybir.dt.float16
    out_d = nc.dram_tensor("out", (P, n_pos * F), out_dt, kind="ExternalOutput")

    S_max = int(max(S))
    OUT_BATCH = 8  # groups per output DMA

    with TileContext(nc) as tc:
        with (
            tc.tile_pool(name="const", bufs=1) as cpool,
            tc.tile_pool(name="chunk", bufs=3) as dpool,
            tc.tile_pool(name="oh", bufs=4) as ohpool,
            tc.tile_pool(name="res", bufs=2) as rpool,
            tc.tile_pool(name="psum", bufs=4, space="PSUM") as ppool,
        ):
            rel_sb = cpool.tile([P, T_core], mm_dt)
            nc.sync.dma_start(rel_sb[:], rel_d[:])
            inv_sb = cpool.tile([P, n_pos], f32)
            nc.sync.dma_start(inv_sb[:], inv_d[:])
            iota_sb = cpool.tile([P, P], mm_dt)
            nc.gpsimd.iota(
                iota_sb[:],
                pattern=[[1, P]],
                base=0,
                channel_multiplier=0,
                allow_small_or_imprecise_dtypes=True,
            )

            IN_BATCH = 6  # groups per input DMA (stream is contiguous)
            t0 = 0
            res = None
            for j0 in range(0, n_pos, IN_BATCH):
                jset = list(range(j0, min(j0 + IN_BATCH, n_pos)))
                Sb = int(sum(int(S[j]) for j in jset))
                chunk = dpool.tile([P, IN_BATCH * S_max * F], mm_dt, tag="chunk")
                nc.sync.dma_start(
                    chunk[:, : Sb * F], data_d[:, t0 * F : (t0 + Sb) * F]
                )
                if MODE == "split":
                    chunk_lo = dpool.tile(
                        [P, IN_BATCH * S_max * F], mm_dt, tag="chunk_lo"
                    )
                    nc.sync.dma_start(
                        chunk_lo[:, : Sb * F], lo_d[:, t0 * F : (t0 + Sb) * F]
                    )
                kb = 0
                for j in jset:
                    Sj = int(S[j])
                    # all Sj one-hots in one DVE op (step-0 broadcast APs)
                    oh = ohpool.tile([P, S_max * P], mm_dt, tag="oh")
                    nc.vector.tensor_tensor(
                        out=oh[:, : Sj * P].rearrange("p (n f) -> p n f", f=P),
                        in0=iota_sb[:, None, :].to_broadcast([P, Sj, P]),
                        in1=rel_sb[:, t0 + kb : t0 + kb + Sj].to_broadcast(
                            [P, Sj, P]
                        ),
                        op=mybir.AluOpType.is_equal,
                    )
                    ps = ppool.tile([P, F], f32)
                    for k_ in range(Sj):
                        k = kb + k_
                        nc.tensor.matmul(
                            ps[:],
                            lhsT=oh[:, k_ * P : (k_ + 1) * P],
                            rhs=chunk[:, k * F : (k + 1) * F],
                            start=(k_ == 0),
                            stop=(k_ == Sj - 1) and MODE != "split",
                        )
                        if MODE == "split":
                            nc.tensor.matmul(
                                ps[:],
                                lhsT=oh[:, k_ * P : (k_ + 1) * P],
                                rhs=chunk_lo[:, k * F : (k + 1) * F],
                                start=False,
                                stop=(k_ == Sj - 1),
                            )
                    jb = j % OUT_BATCH
                    if jb == 0:
                        res = rpool.tile([P, OUT_BATCH * F], out_dt, tag="res")
                    nc.scalar.activation(
                        res[:, jb * F : (jb + 1) * F],
                        ps[:],
                        mybir.ActivationFunctionType.Copy,
                        scale=inv_sb[:, j : j + 1],
                    )
                    if jb == OUT_BATCH - 1 or j == n_pos - 1:
                        lo = (j - jb) * F
                        nc.sync.dma_start(
                            out_d[:, lo : (j + 1) * F], res[:, : (jb + 1) * F]
                        )
                    kb += Sj
                t0 += Sb

    _split_waits(nc)
    return nc


# ---------------------------------------------------------------------------
def kernel(data, tokens_to_node_map, W=None, b=None, scoring=None):
    global LAST_RESULTS
    from concourse import bass_utils

    in_maps, meta = _preprocess(data, tokens_to_node_map)
    nc = _build_kernel(meta["S"], meta["n_pos"], meta["T_core"])

    kwargs = {}
    if TRACE and _enable_profiling():
        kwargs["trace"] = True
    res = None
    for attempt in range(3):
        try:
            res = bass_utils.run_bass_kernel_spmd(
                nc, in_maps, core_ids=list(range(N_CORES)), **kwargs
            )
            break
        except Exception:
            if attempt == 2:
                raise
            kwargs.pop("trace", None)  # drop profiling on retry
    LAST_RESULTS = res

    n_pos = meta["n_pos"]
    assign = meta["assign"]
    out = np.zeros((NUM_NODES, F), np.float32)
    for c in range(N_CORES):
        oc = res.results[c]["out"]
        for j in range(n_pos):
            g = int(assign[c, j])
            if g < 0:
                continue
            base = P * g
            hi = min(P, NUM_NODES - base)
            out[base : base + hi] = oc[:hi, j * F : (j + 1) * F]
    return out
